# revision 1
# baseline (speedup 1.0000x reference)
"""Contrastive loss (SimCLR-style NT-Xent, faithful variant) on 8 Trainium2 cores.

Problem: x1, x2 [4096, 256] f32.  z = normalize(concat(x1, x2)) [8192, 256];
sim = z @ z.T; pos = diag(sim, +4096) used for both halves;
den_g = sum_j exp(mask_offdiag * sim_gj / tau)  (diag contributes exp(0)=1);
loss = mean(log(den) - pos_pairs/tau).

Sharding: each core c owns 1024 rows (rows [c*1024, (c+1)*1024) of the 8192).
Every core receives the full feature-major x (xT, f32) plus its own row-block
and the paired row-block pre-sliced (avoids per-core program differences in a
single SPMD program).  Each core normalizes on device, computes its [1024, 8192]
row-block of sim as a bf16 GEMM (fp32 accumulate), applies exp(x/tau) fused with
row-sum accumulation on the scalar engine, corrects the diagonal term
arithmetically (+1 - exp(selfsim/tau)), adds the positive-pair term, and emits
one partial-loss scalar.  Host sums the 8 scalars and divides by 2N.

bf16 GEMM end-to-end loss rel-err vs fp32 reference measured at ~7e-6 (numpy).
"""

import numpy as np

import concourse.bass as bass
import concourse.tile as tile
from concourse import bacc, mybir

F32 = mybir.dt.float32
F32R = mybir.dt.float32r
BF16 = mybir.dt.bfloat16
AF = mybir.ActivationFunctionType
ALU = mybir.AluOpType
AX = mybir.AxisListType
PSUM = bass.MemorySpace.PSUM

N = 4096
TWO_N = 2 * N
D = 256
RPC = TWO_N // 8          # rows per core = 1024
TAU_INV = 10.0            # 1/tau
M_TILES = RPC // 128      # 8 row tiles per core
NB = TWO_N // 2048        # 4 column superblocks of 2048


def build_nc(nc=None):
    if nc is None:
        nc = bacc.Bacc("TRN2", target_bir_lowering=False, debug=False)

    xt = [
        nc.declare_dram_parameter(f"xt{k}", [128, TWO_N], F32, isOutput=False)
        for k in range(2)
    ]
    xa = [
        nc.declare_dram_parameter(f"xa{k}", [128, RPC], F32, isOutput=False)
        for k in range(2)
    ]
    xb = [
        nc.declare_dram_parameter(f"xb{k}", [128, RPC], F32, isOutput=False)
        for k in range(2)
    ]
    out_d = nc.declare_dram_parameter("out", [1, 1], F32, isOutput=True)

    with tile.TileContext(nc) as tc:
        with (
            tc.tile_pool(name="const", bufs=1) as cpool,
            tc.tile_pool(name="xt", bufs=1) as xt_pool,
            tc.tile_pool(name="zt", bufs=1) as zt_pool,
            tc.tile_pool(name="ab", bufs=1) as ab_pool,
            tc.tile_pool(name="rows", bufs=1) as row_pool,
            tc.tile_pool(name="xsq", bufs=3) as xsq_pool,
            tc.tile_pool(name="fin", bufs=1) as fin_pool,
        ):
            ones_col32 = cpool.tile([128, 1], F32, name="ones_col32", tag="ones_col32")
            nc.vector.memset(ones_col32[:], 1.0)
            ones_row32 = cpool.tile([1, 128], F32, name="ones_row32", tag="ones_row32")
            nc.vector.memset(ones_row32[:], 1.0)
            ones_col = cpool.tile([128, 1], F32R, name="ones_col", tag="ones_col")
            nc.vector.tensor_copy(ones_col[:], ones_col32[:])
            ones_row = cpool.tile([1, 128], F32R, name="ones_row", tag="ones_row")
            nc.vector.tensor_copy(ones_row[:], ones_row32[:])

            # persistent SBUF tensors
            xt_t = [
                xt_pool.tile([128, TWO_N], F32, name=f"xts{k}", tag=f"xts{k}")
                for k in range(2)
            ]
            zt_t = [
                zt_pool.tile([128, TWO_N], BF16, name=f"zts{k}", tag=f"zts{k}")
                for k in range(2)
            ]
            xa_t = [
                ab_pool.tile([128, RPC], F32, name=f"xas{k}", tag=f"xas{k}")
                for k in range(2)
            ]
            xb_t = [
                ab_pool.tile([128, RPC], F32, name=f"xbs{k}", tag=f"xbs{k}")
                for k in range(2)
            ]
            za_t = [
                ab_pool.tile([128, RPC], BF16, name=f"zas{k}", tag=f"zas{k}")
                for k in range(2)
            ]
            zb_t = [
                ab_pool.tile([128, RPC], BF16, name=f"zbs{k}", tag=f"zbs{k}")
                for k in range(2)
            ]

            den_acc = fin_pool.tile(
                [128, M_TILES * NB], F32, name="den_acc", tag="den_acc"
            )
            selfexp_t = fin_pool.tile(
                [128, M_TILES], F32, name="selfexp_t", tag="selfexp_t"
            )
            possum = fin_pool.tile([1, 1], F32, name="possum", tag="possum")
            selfexp_row = row_pool.tile(
                [1, RPC], F32, name="selfexp_row", tag="selfexp_row"
            )

            # ---- input DMAs ----
            for k in range(2):
                nc.sync.dma_start(xa_t[k][:], xa[k][:])
                nc.sync.dma_start(xb_t[k][:], xb[k][:])
            for blk in range(8):
                cs = slice(blk * 1024, (blk + 1) * 1024)
                for k in range(2):
                    nc.sync.dma_start(xt_t[k][:, cs], xt[k][:, cs])

            # ---- prep phase (PSUM pools scoped) ----
            with (
                tc.tile_pool(name="ssp", bufs=2, space=PSUM) as ss_pool,
                tc.tile_pool(name="bcp", bufs=2, space=PSUM) as bc_pool,
                tc.tile_pool(name="pselfp", bufs=1, space=PSUM) as ps_pool,
            ):

                def normalize(src_t, dst_t, width, rs_tag, sq_engine):
                    """Normalize embedding columns: dst = src * rsqrt(colsum(src^2)).

                    src_t/dst_t: two [128, width] chunks (f32 -> bf16).
                    Processes in 1024-column blocks for pipelining.
                    """
                    for blk in range(width // 1024):
                        bs = slice(blk * 1024, (blk + 1) * 1024)
                        xsq = [
                            xsq_pool.tile(
                                [128, 1024], F32R,
                                name=f"xsq{k}_{rs_tag}", tag=f"xsq{k}_{rs_tag}",
                                bufs=(2 if width > RPC else 1),
                            )
                            for k in range(2)
                        ]
                        for k in range(2):
                            # alternate engines so neither rate-limits the
                            # downstream sumsq/normalize pipeline
                            eng = sq_engine if (blk + k) % 2 == 0 else nc.vector
                            eng.tensor_mul(
                                xsq[k][:], src_t[k][:, bs], src_t[k][:, bs]
                            )
                        for j in range(2):
                            js = slice(blk * 1024 + j * 512, blk * 1024 + (j + 1) * 512)
                            jl = slice(j * 512, (j + 1) * 512)
                            ss = ss_pool.tile([1, 512], F32, name="ss", tag="ss")
                            for k in range(2):
                                nc.tensor.matmul(
                                    ss[:],
                                    ones_col[:],
                                    xsq[k][:, jl],
                                    start=(k == 0),
                                    stop=(k == 1),
                                )
                            ln_blk = row_pool.tile(
                                [1, 512], F32R, name="ln_blk", tag="ln_blk", bufs=2
                            )
                            nc.scalar.activation(ln_blk[:], ss[:], AF.Ln)
                            bc = bc_pool.tile([128, 512], F32, name="bc", tag="bc")
                            nc.tensor.matmul(
                                bc[:],
                                ones_row[:],
                                ln_blk[:],
                                start=True,
                                stop=True,
                            )
                            rbc = row_pool.tile(
                                [128, 512], F32, name="rbc", tag="rbc", bufs=2
                            )
                            # rsqrt(ss) broadcast: exp(-0.5 * ln(ss))
                            nc.scalar.activation(rbc[:], bc[:], AF.Exp, scale=-0.5)
                            for k in range(2):
                                nc.vector.tensor_mul(
                                    dst_t[k][:, js], src_t[k][:, js], rbc[:]
                                )

                # small blocks first: za/zb ready early for the main GEMM lhsT
                normalize(xa_t, za_t, RPC, "a", nc.vector)
                normalize(xb_t, zb_t, RPC, "b", nc.vector)

                # pos & selfsim: per-row dot products via elementwise mul +
                # ones-matmul partition reduction -> [1, RPC] rows
                pos_ps = ps_pool.tile([1, RPC], F32, name="pos", tag="pos")
                selfs_ps = ps_pool.tile([1, RPC], F32, name="selfs", tag="selfs")
                prod_a = [
                    xsq_pool.tile(
                        [128, RPC], F32R, name=f"prod_a{k}", tag=f"prod_a{k}", bufs=1
                    )
                    for k in range(2)
                ]
                prod_s = [
                    xsq_pool.tile(
                        [128, RPC], F32R, name=f"prod_s{k}", tag=f"prod_s{k}", bufs=1
                    )
                    for k in range(2)
                ]
                for k in range(2):
                    nc.vector.tensor_mul(prod_a[k][:], za_t[k][:], zb_t[k][:])
                    nc.vector.tensor_mul(prod_s[k][:], za_t[k][:], za_t[k][:])
                for j in range(RPC // 512):
                    js = slice(j * 512, (j + 1) * 512)
                    for k in range(2):
                        nc.tensor.matmul(
                            pos_ps[0:1, js],
                            ones_col[:],
                            prod_a[k][:, js],
                            start=(k == 0),
                            stop=(k == 1),
                        )
                    for k in range(2):
                        nc.tensor.matmul(
                            selfs_ps[0:1, js],
                            ones_col[:],
                            prod_s[k][:, js],
                            start=(k == 0),
                            stop=(k == 1),
                        )
                nc.vector.tensor_reduce(possum[:], pos_ps[:], axis=AX.X, op=ALU.add)
                nc.scalar.activation(selfexp_row[:], selfs_ps[:], AF.Exp, scale=TAU_INV)
                # transpose [1, 1024] -> [128, 8] (row g = m*128 + p -> [p, m])
                # via a DRAM bounce (SBUF->SBUF transposing APs don't balance)
                with tc.tile_pool(name="dram", bufs=1, space="DRAM") as dram_pool:
                    se_dram = dram_pool.tile(
                        [1, RPC], F32, name="se_dram", tag="se_dram"
                    )
                    nc.sync.dma_start(se_dram[:], selfexp_row[:])
                    nc.sync.dma_start(
                        selfexp_t[:],
                        se_dram[0:1, :].rearrange("o (m p) -> (o p) m", p=128),
                    )

                # full zT (the GEMM rhs); squares on the otherwise-idle gpsimd
                normalize(xt_t, zt_t, TWO_N, "t", nc.gpsimd)

            # ---- main loop: sim row-block GEMM + fused exp/rowsum ----
            # nb outer so the GEMM starts as soon as the first 2048 columns of
            # zT are normalized.
            with tc.tile_pool(name="simp", bufs=2, space=PSUM) as sim_pool:
                for nb in range(NB):
                    for m in range(M_TILES):
                        ms = slice(m * 128, (m + 1) * 128)
                        st = sim_pool.tile([128, 2048], F32, name="sim", tag="sim")
                        for k in range(2):
                            for j4 in range(4):
                                js = slice(j4 * 512, (j4 + 1) * 512)
                                cs = slice(
                                    nb * 2048 + j4 * 512, nb * 2048 + (j4 + 1) * 512
                                )
                                nc.tensor.matmul(
                                    st[:, js],
                                    za_t[k][:, ms],
                                    zt_t[k][:, cs],
                                    start=(k == 0),
                                    stop=(k == 1),
                                )
                        idx = m * NB + nb
                        nc.scalar.activation(
                            st[:],
                            st[:],
                            AF.Exp,
                            scale=TAU_INV,
                            accum_out=den_acc[:, idx : idx + 1],
                        )

            # ---- finalize ----
            with tc.tile_pool(name="finp", bufs=1, space=PSUM) as fpsum:
                den8 = fin_pool.tile([128, M_TILES], F32, name="den8", tag="den8")
                nc.vector.tensor_reduce(
                    den8[:],
                    den_acc[:].rearrange("p (m n) -> p m n", n=NB),
                    axis=AX.X,
                    op=ALU.add,
                )
                denc = fin_pool.tile([128, M_TILES], F32, name="denc", tag="denc")
                # (den8 + 1) - selfexp : diag contributed exp(selfsim/tau), the
                # reference wants exp(0)=1 there instead.
                nc.vector.scalar_tensor_tensor(
                    denc[:],
                    in0=den8[:],
                    scalar=1.0,
                    in1=selfexp_t[:],
                    op0=ALU.add,
                    op1=ALU.subtract,
                )
                logden = fin_pool.tile([128, M_TILES], F32, name="logden", tag="logden")
                nc.scalar.activation(logden[:], denc[:], AF.Ln)
                red = fin_pool.tile([128, 1], F32, name="red", tag="red")
                nc.vector.tensor_reduce(red[:], logden[:], axis=AX.X, op=ALU.add)
                tot_ps = fpsum.tile([1, 1], F32, name="tot", tag="tot")
                nc.tensor.matmul(
                    tot_ps[:],
                    ones_col32[:],
                    red[:],
                    start=True,
                    stop=True,
                )
                res = fin_pool.tile([1, 1], F32, name="res", tag="res")
                # res = possum * (-1/tau) + sum(log den)
                nc.vector.scalar_tensor_tensor(
                    res[:],
                    in0=possum[:],
                    scalar=-TAU_INV,
                    in1=tot_ps[:],
                    op0=ALU.mult,
                    op1=ALU.add,
                )
                nc.sync.dma_start(out_d[:], res[:])

    nc.compile()
    return nc


_NC = None


def _get_nc():
    global _NC
    if _NC is None:
        _NC = build_nc()
    return _NC


def make_in_maps(x1, x2):
    x1 = np.asarray(x1, dtype=np.float32)
    x2 = np.asarray(x2, dtype=np.float32)
    x = np.concatenate([x1, x2], axis=0)              # [8192, 256]
    xT = np.ascontiguousarray(x.T)                    # [256, 8192]
    xt0, xt1 = xT[:128], xT[128:]
    in_maps = []
    for c in range(8):
        cb = c * RPC
        pb = (cb + N) % TWO_N
        in_maps.append(
            {
                "xt0": xt0,
                "xt1": xt1,
                "xa0": np.ascontiguousarray(xt0[:, cb : cb + RPC]),
                "xa1": np.ascontiguousarray(xt1[:, cb : cb + RPC]),
                "xb0": np.ascontiguousarray(xt0[:, pb : pb + RPC]),
                "xb1": np.ascontiguousarray(xt1[:, pb : pb + RPC]),
            }
        )
    return in_maps


def _run(x1, x2, trace=False, tmpdir=None):
    from concourse.bass_utils import run_bass_kernel_spmd

    nc = _get_nc()
    in_maps = make_in_maps(x1, x2)
    res = run_bass_kernel_spmd(
        nc, in_maps, list(range(8)), trace=trace, tmpdir=tmpdir
    )
    total = sum(float(res.results[c]["out"][0, 0]) for c in range(8))
    loss = np.asarray(np.float32(total / TWO_N))
    return loss, res


def kernel(x1, x2):
    loss, _ = _run(x1, x2)
    return loss



# revision 2
# speedup vs baseline: 1.1297x; 1.1297x over previous
"""Contrastive loss (SimCLR-style NT-Xent, faithful variant) on 8 Trainium2 cores.

Problem: x1, x2 [4096, 256] f32.  z = normalize(concat(x1, x2)) [8192, 256];
sim = z @ z.T; pos = diag(sim, +4096) used for both halves;
den_g = sum_j exp(mask_offdiag * sim_gj / tau)  (diag contributes exp(0)=1);
loss = mean(log(den) - pos_pairs/tau).

Sharding: each core c owns 1024 rows (rows [c*1024, (c+1)*1024) of the 8192).
Every core receives the full feature-major x (xT, f32) plus its own row-block
and the paired row-block pre-sliced (avoids per-core program differences in a
single SPMD program).  Each core normalizes on device, computes its [1024, 8192]
row-block of sim as a bf16 GEMM (fp32 accumulate), applies exp(x/tau) fused with
row-sum accumulation on the scalar engine, corrects the diagonal term
arithmetically (+1 - exp(selfsim/tau)), adds the positive-pair term, and emits
one partial-loss scalar.  Host sums the 8 scalars and divides by 2N.

bf16 GEMM end-to-end loss rel-err vs fp32 reference measured at ~7e-6 (numpy).
"""

import numpy as np

import concourse.bass as bass
import concourse.tile as tile
from concourse import bacc, mybir

F32 = mybir.dt.float32
F32R = mybir.dt.float32r
BF16 = mybir.dt.bfloat16
AF = mybir.ActivationFunctionType
ALU = mybir.AluOpType
AX = mybir.AxisListType
PSUM = bass.MemorySpace.PSUM

N = 4096
TWO_N = 2 * N
D = 256
RPC = TWO_N // 8          # rows per core = 1024
TAU_INV = 10.0            # 1/tau
M_TILES = RPC // 128      # 8 row tiles per core
NB = TWO_N // 2048        # 4 column superblocks of 2048


def _patch_act_tables():
    """Make ln and exp resolve to the one table set that holds BOTH.

    The stock set-picker chooses the first set containing each function
    (ln -> natural_log, exp -> exp_and_others), so a kernel alternating
    ln/exp reloads activation tables on every switch (~2.7us each, ~25
    loads in this kernel).  Stripping ln/exp from the single-function
    sets leaves natural_log_exp_and_others as the only candidate for
    both, so the whole kernel runs on one ACT_TABLE_LOAD.  Set indices
    are preserved (entries are edited in place, not removed).
    """
    import concourse.bacc as _bacc
    import concourse.hw_specs as _hw

    orig = _hw.get_activation_tables

    def patched(arch):
        tables = dict(orig(arch))
        ln = mybir.ActivationFunctionType.Ln
        exp = mybir.ActivationFunctionType.Exp
        out = {}
        for name, funcs in tables.items():
            if name != "natural_log_exp_and_others" and (
                ln in funcs or exp in funcs
            ):
                funcs = funcs - {ln, exp}
            out[name] = funcs
        return out

    _bacc.get_activation_tables = patched


def build_nc(nc=None):
    _patch_act_tables()
    if nc is None:
        nc = bacc.Bacc("TRN2", target_bir_lowering=False, debug=False)

    xt = [
        nc.declare_dram_parameter(f"xt{k}", [128, TWO_N], F32, isOutput=False)
        for k in range(2)
    ]
    xa = [
        nc.declare_dram_parameter(f"xa{k}", [128, RPC], F32, isOutput=False)
        for k in range(2)
    ]
    xb = [
        nc.declare_dram_parameter(f"xb{k}", [128, RPC], F32, isOutput=False)
        for k in range(2)
    ]
    out_d = nc.declare_dram_parameter("out", [1, 1], F32, isOutput=True)

    with tile.TileContext(nc) as tc:
        with (
            tc.tile_pool(name="const", bufs=1) as cpool,
            tc.tile_pool(name="xt", bufs=1) as xt_pool,
            tc.tile_pool(name="zt", bufs=1) as zt_pool,
            tc.tile_pool(name="ab", bufs=1) as ab_pool,
            tc.tile_pool(name="rows", bufs=1) as row_pool,
            tc.tile_pool(name="xsq", bufs=3) as xsq_pool,
            tc.tile_pool(name="fin", bufs=1) as fin_pool,
        ):
            ones_col32 = cpool.tile([128, 1], F32, name="ones_col32", tag="ones_col32")
            nc.vector.memset(ones_col32[:], 1.0)
            ones_row32 = cpool.tile([1, 128], F32, name="ones_row32", tag="ones_row32")
            nc.vector.memset(ones_row32[:], 1.0)
            ones_col = cpool.tile([128, 1], F32R, name="ones_col", tag="ones_col")
            nc.vector.tensor_copy(ones_col[:], ones_col32[:])
            ones_row = cpool.tile([1, 128], F32R, name="ones_row", tag="ones_row")
            nc.vector.tensor_copy(ones_row[:], ones_row32[:])

            # persistent SBUF tensors
            xt_t = [
                xt_pool.tile([128, TWO_N], F32, name=f"xts{k}", tag=f"xts{k}")
                for k in range(2)
            ]
            zt_t = [
                zt_pool.tile([128, TWO_N], BF16, name=f"zts{k}", tag=f"zts{k}")
                for k in range(2)
            ]
            xa_t = [
                ab_pool.tile([128, RPC], F32, name=f"xas{k}", tag=f"xas{k}")
                for k in range(2)
            ]
            xb_t = [
                ab_pool.tile([128, RPC], F32, name=f"xbs{k}", tag=f"xbs{k}")
                for k in range(2)
            ]
            za_t = [
                ab_pool.tile([128, RPC], BF16, name=f"zas{k}", tag=f"zas{k}")
                for k in range(2)
            ]
            zb_t = [
                ab_pool.tile([128, RPC], BF16, name=f"zbs{k}", tag=f"zbs{k}")
                for k in range(2)
            ]

            den_acc = fin_pool.tile(
                [128, M_TILES * NB], F32, name="den_acc", tag="den_acc"
            )
            selfexp_t = fin_pool.tile(
                [128, M_TILES], F32, name="selfexp_t", tag="selfexp_t"
            )
            possum = fin_pool.tile([1, 1], F32, name="possum", tag="possum")
            selfexp_row = row_pool.tile(
                [1, RPC], F32, name="selfexp_row", tag="selfexp_row"
            )

            # ---- input DMAs ----
            for k in range(2):
                nc.sync.dma_start(xa_t[k][:], xa[k][:])
                nc.sync.dma_start(xb_t[k][:], xb[k][:])
            for blk in range(8):
                cs = slice(blk * 1024, (blk + 1) * 1024)
                for k in range(2):
                    nc.sync.dma_start(xt_t[k][:, cs], xt[k][:, cs])

            # ---- prep phase (PSUM pools scoped) ----
            with (
                tc.tile_pool(name="ssp", bufs=2, space=PSUM) as ss_pool,
                tc.tile_pool(name="bcp", bufs=2, space=PSUM) as bc_pool,
                tc.tile_pool(name="pselfp", bufs=1, space=PSUM) as ps_pool,
            ):

                def normalize(src_t, dst_t, width, rs_tag, sq_engine):
                    """Normalize embedding columns: dst = src * rsqrt(colsum(src^2)).

                    src_t/dst_t: two [128, width] chunks (f32 -> bf16).
                    Processes in 1024-column blocks for pipelining.
                    """
                    for blk in range(width // 1024):
                        bs = slice(blk * 1024, (blk + 1) * 1024)
                        xsq = [
                            xsq_pool.tile(
                                [128, 1024], F32R,
                                name=f"xsq{k}_{rs_tag}", tag=f"xsq{k}_{rs_tag}",
                                bufs=(2 if width > RPC else 1),
                            )
                            for k in range(2)
                        ]
                        for k in range(2):
                            # alternate engines so neither rate-limits the
                            # downstream sumsq/normalize pipeline
                            eng = sq_engine if (blk + k) % 2 == 0 else nc.vector
                            eng.tensor_mul(
                                xsq[k][:], src_t[k][:, bs], src_t[k][:, bs]
                            )
                        for j in range(2):
                            js = slice(blk * 1024 + j * 512, blk * 1024 + (j + 1) * 512)
                            jl = slice(j * 512, (j + 1) * 512)
                            ss = ss_pool.tile([1, 512], F32, name="ss", tag="ss")
                            for k in range(2):
                                nc.tensor.matmul(
                                    ss[:],
                                    ones_col[:],
                                    xsq[k][:, jl],
                                    start=(k == 0),
                                    stop=(k == 1),
                                )
                            ln_blk = row_pool.tile(
                                [1, 512], F32R, name="ln_blk", tag="ln_blk", bufs=2
                            )
                            nc.scalar.activation(ln_blk[:], ss[:], AF.Ln)
                            bc = bc_pool.tile([128, 512], F32, name="bc", tag="bc")
                            nc.tensor.matmul(
                                bc[:],
                                ones_row[:],
                                ln_blk[:],
                                start=True,
                                stop=True,
                            )
                            rbc = row_pool.tile(
                                [128, 512], F32, name="rbc", tag="rbc", bufs=2
                            )
                            # rsqrt(ss) broadcast: exp(-0.5 * ln(ss))
                            nc.scalar.activation(rbc[:], bc[:], AF.Exp, scale=-0.5)
                            for k in range(2):
                                nc.vector.tensor_mul(
                                    dst_t[k][:, js], src_t[k][:, js], rbc[:]
                                )

                # small blocks first: za/zb ready early for the main GEMM lhsT
                normalize(xa_t, za_t, RPC, "a", nc.vector)
                normalize(xb_t, zb_t, RPC, "b", nc.vector)

                # pos & selfsim: per-row dot products via elementwise mul +
                # ones-matmul partition reduction -> [1, RPC] rows
                pos_ps = ps_pool.tile([1, RPC], F32, name="pos", tag="pos")
                selfs_ps = ps_pool.tile([1, RPC], F32, name="selfs", tag="selfs")
                prod_a = [
                    xsq_pool.tile(
                        [128, RPC], F32R, name=f"prod_a{k}", tag=f"prod_a{k}", bufs=1
                    )
                    for k in range(2)
                ]
                prod_s = [
                    xsq_pool.tile(
                        [128, RPC], F32R, name=f"prod_s{k}", tag=f"prod_s{k}", bufs=1
                    )
                    for k in range(2)
                ]
                for k in range(2):
                    nc.vector.tensor_mul(prod_a[k][:], za_t[k][:], zb_t[k][:])
                    nc.vector.tensor_mul(prod_s[k][:], za_t[k][:], za_t[k][:])
                for j in range(RPC // 512):
                    js = slice(j * 512, (j + 1) * 512)
                    for k in range(2):
                        nc.tensor.matmul(
                            pos_ps[0:1, js],
                            ones_col[:],
                            prod_a[k][:, js],
                            start=(k == 0),
                            stop=(k == 1),
                        )
                    for k in range(2):
                        nc.tensor.matmul(
                            selfs_ps[0:1, js],
                            ones_col[:],
                            prod_s[k][:, js],
                            start=(k == 0),
                            stop=(k == 1),
                        )
                nc.vector.tensor_reduce(possum[:], pos_ps[:], axis=AX.X, op=ALU.add)
                nc.scalar.activation(selfexp_row[:], selfs_ps[:], AF.Exp, scale=TAU_INV)
                # transpose [1, 1024] -> [128, 8] (row g = m*128 + p -> [p, m])
                # via a DRAM bounce (SBUF->SBUF transposing APs don't balance)
                with tc.tile_pool(name="dram", bufs=1, space="DRAM") as dram_pool:
                    se_dram = dram_pool.tile(
                        [1, RPC], F32, name="se_dram", tag="se_dram"
                    )
                    nc.sync.dma_start(se_dram[:], selfexp_row[:])
                    nc.sync.dma_start(
                        selfexp_t[:],
                        se_dram[0:1, :].rearrange("o (m p) -> (o p) m", p=128),
                    )

                # full zT (the GEMM rhs); squares on the otherwise-idle gpsimd
                normalize(xt_t, zt_t, TWO_N, "t", nc.gpsimd)

            # ---- main loop: sim row-block GEMM + fused exp/rowsum ----
            # nb outer so the GEMM starts as soon as the first 2048 columns of
            # zT are normalized.
            with tc.tile_pool(name="simp", bufs=2, space=PSUM) as sim_pool:
                for nb in range(NB):
                    for m in range(M_TILES):
                        ms = slice(m * 128, (m + 1) * 128)
                        st = sim_pool.tile([128, 2048], F32, name="sim", tag="sim")
                        for k in range(2):
                            for j4 in range(4):
                                js = slice(j4 * 512, (j4 + 1) * 512)
                                cs = slice(
                                    nb * 2048 + j4 * 512, nb * 2048 + (j4 + 1) * 512
                                )
                                nc.tensor.matmul(
                                    st[:, js],
                                    za_t[k][:, ms],
                                    zt_t[k][:, cs],
                                    start=(k == 0),
                                    stop=(k == 1),
                                )
                        idx = m * NB + nb
                        nc.scalar.activation(
                            st[:],
                            st[:],
                            AF.Exp,
                            scale=TAU_INV,
                            accum_out=den_acc[:, idx : idx + 1],
                        )

            # ---- finalize ----
            with tc.tile_pool(name="finp", bufs=1, space=PSUM) as fpsum:
                den8 = fin_pool.tile([128, M_TILES], F32, name="den8", tag="den8")
                nc.vector.tensor_reduce(
                    den8[:],
                    den_acc[:].rearrange("p (m n) -> p m n", n=NB),
                    axis=AX.X,
                    op=ALU.add,
                )
                denc = fin_pool.tile([128, M_TILES], F32, name="denc", tag="denc")
                # (den8 + 1) - selfexp : diag contributed exp(selfsim/tau), the
                # reference wants exp(0)=1 there instead.
                nc.vector.scalar_tensor_tensor(
                    denc[:],
                    in0=den8[:],
                    scalar=1.0,
                    in1=selfexp_t[:],
                    op0=ALU.add,
                    op1=ALU.subtract,
                )
                logden = fin_pool.tile([128, M_TILES], F32, name="logden", tag="logden")
                nc.scalar.activation(logden[:], denc[:], AF.Ln)
                red = fin_pool.tile([128, 1], F32, name="red", tag="red")
                nc.vector.tensor_reduce(red[:], logden[:], axis=AX.X, op=ALU.add)
                tot_ps = fpsum.tile([1, 1], F32, name="tot", tag="tot")
                nc.tensor.matmul(
                    tot_ps[:],
                    ones_col32[:],
                    red[:],
                    start=True,
                    stop=True,
                )
                res = fin_pool.tile([1, 1], F32, name="res", tag="res")
                # res = possum * (-1/tau) + sum(log den)
                nc.vector.scalar_tensor_tensor(
                    res[:],
                    in0=possum[:],
                    scalar=-TAU_INV,
                    in1=tot_ps[:],
                    op0=ALU.mult,
                    op1=ALU.add,
                )
                nc.sync.dma_start(out_d[:], res[:])

    nc.compile()
    return nc


_NC = None


def _get_nc():
    global _NC
    if _NC is None:
        _NC = build_nc()
    return _NC


def make_in_maps(x1, x2):
    x1 = np.asarray(x1, dtype=np.float32)
    x2 = np.asarray(x2, dtype=np.float32)
    x = np.concatenate([x1, x2], axis=0)              # [8192, 256]
    xT = np.ascontiguousarray(x.T)                    # [256, 8192]
    xt0, xt1 = xT[:128], xT[128:]
    in_maps = []
    for c in range(8):
        cb = c * RPC
        pb = (cb + N) % TWO_N
        in_maps.append(
            {
                "xt0": xt0,
                "xt1": xt1,
                "xa0": np.ascontiguousarray(xt0[:, cb : cb + RPC]),
                "xa1": np.ascontiguousarray(xt1[:, cb : cb + RPC]),
                "xb0": np.ascontiguousarray(xt0[:, pb : pb + RPC]),
                "xb1": np.ascontiguousarray(xt1[:, pb : pb + RPC]),
            }
        )
    return in_maps


def _run(x1, x2, trace=False, tmpdir=None):
    from concourse.bass_utils import run_bass_kernel_spmd

    nc = _get_nc()
    in_maps = make_in_maps(x1, x2)
    res = run_bass_kernel_spmd(
        nc, in_maps, list(range(8)), trace=trace, tmpdir=tmpdir
    )
    total = sum(float(res.results[c]["out"][0, 0]) for c in range(8))
    loss = np.asarray(np.float32(total / TWO_N))
    return loss, res


def kernel(x1, x2):
    loss, _ = _run(x1, x2)
    return loss

